# revision 9
# baseline (speedup 1.0000x reference)
"""v7: attention head on 8 trn2 NeuronCores, no collectives.

Sharding: core c handles batch b=c//2 and K/V-half j=c%2. Each core computes
unnormalized attention of the batch's FULL query block (2048 rows) against its
1024-row K/V half; softmax numerator/denominator halves combine linearly on
the host (out = (numA+numB)/(denA+denB)).

Host-side preprocessing (layout/dtype only, no model FLOPs): q/k/v cast to
bf16 and pre-transposed to [H, seq]; the 1/8 score scale folded into Wq/bq.

Device: kT + all four q blocks stream on the sync HWDGE ring while vT rides
the scalar HWDGE ring in parallel; PE is pre-warmed with f32 identity matmuls
so projections run at full clock; scores in [k, q] layout; exp on ACT;
attnV with V-natural stationary carrying a ones column (row 64 = softmax
denominator); per-qn epilogue DMAs the unnormalized [65, 512] slices out.
"""

import sys

if "/opt/trn_rl_repo" not in sys.path:
    sys.path.insert(0, "/opt/trn_rl_repo")

import numpy as np
import ml_dtypes

N, L, H, D = 4, 2048, 1024, 64
NCORES = 8
HC = H // 128  # 8 h-chunks
KH = L // 2  # 1024 rows of K/V per core
KC = KH // 128  # 8 k-chunks per core


def build_bass():
    import concourse.mybir as mybir
    from concourse import bacc
    from concourse.masks import make_identity
    from concourse.tile import TileContext

    f32 = mybir.dt.float32
    bf16 = mybir.dt.bfloat16
    AF = mybir.ActivationFunctionType
    Alu = mybir.AluOpType

    nc = bacc.Bacc("TRN2", target_bir_lowering=False, debug=False)
    qt_d = nc.dram_tensor("qt", [128, 16 * H], bf16, kind="ExternalInput").ap()
    kt_d = nc.dram_tensor("kt", [128, 8 * H], bf16, kind="ExternalInput").ap()
    vt_d = nc.dram_tensor("vt", [128, 8 * H], bf16, kind="ExternalInput").ap()
    w_d = nc.dram_tensor("w", [128, 3 * HC * D], bf16, kind="ExternalInput").ap()
    b_d = nc.dram_tensor("b", [D, 3], f32, kind="ExternalInput").ap()
    out_d = nc.dram_tensor("out", [D + 1, L], f32, kind="ExternalOutput").ap()

    with TileContext(nc) as tc:
        with (
            tc.tile_pool(name="io", bufs=1) as io_pool,
            tc.tile_pool(name="proj", bufs=1) as proj_pool,
            tc.tile_pool(name="e", bufs=12) as e_pool,
            tc.tile_pool(name="ps", bufs=2, space="PSUM") as ps_pool,
            tc.tile_pool(name="acc", bufs=1, space="PSUM") as acc_pool,
        ):
            w_sb = io_pool.tile([128, 3 * HC * D], bf16, tag="w")
            b_sb = io_pool.tile([D, 3], f32, tag="b")
            kt_sb = io_pool.tile([128, 8 * H], bf16, tag="kt")
            vt_sb = io_pool.tile([128, 8 * H], bf16, tag="vt")
            qt_sb = io_pool.tile([128, 16 * H], bf16, tag="qt")
            # sync ring: w, b, kT halves, qn0..qn3.  scalar ring: vT halves
            # (runs in parallel, sharing HBM bandwidth).
            nc.sync.dma_start(out=w_sb[:], in_=w_d[:])
            nc.sync.dma_start(out=b_sb[:], in_=b_d[:])
            for i in range(2):
                nc.sync.dma_start(
                    out=kt_sb[:, i * 4 * KH : (i + 1) * 4 * KH],
                    in_=kt_d[:, i * 4 * KH : (i + 1) * 4 * KH],
                )
            for qn in range(4):
                nc.sync.dma_start(
                    out=qt_sb[:, qn * 4 * H : (qn + 1) * 4 * H],
                    in_=qt_d[:, qn * 4 * H : (qn + 1) * 4 * H],
                )
            for i in range(2):
                nc.scalar.dma_start(
                    out=vt_sb[:, i * 4 * KH : (i + 1) * 4 * KH],
                    in_=vt_d[:, i * 4 * KH : (i + 1) * 4 * KH],
                )

            identf = io_pool.tile([128, 128], f32, tag="identf")
            make_identity(nc, identf[:])

            kprojT = proj_pool.tile([D, KH], bf16, tag="kprojT")
            qprojT = proj_pool.tile([D, L], bf16, tag="qprojT")
            vp = proj_pool.tile([128, KC * (D + 1)], bf16, tag="vp")
            outT_sb = proj_pool.tile([D + 1, L], f32, tag="outT")
            vhalf = proj_pool.tile([D, KH], f32, tag="vhalf")
            vprojT = proj_pool.tile([D, KH], f32, tag="vprojT")

            # ---- K projection (PE warmup matmuls share its psum tile) ----
            psk = ps_pool.tile([128, 1024], f32, tag="ps", name="psk")
            for _ in range(10):
                nc.tensor.matmul(
                    psk[:, 0:128], identf[:], identf[:], start=True, stop=True,
                )
            for hc in range(HC):
                wslice = w_sb[:, (HC + hc) * D : (HC + hc + 1) * D]
                for sn in range(2):
                    nc.tensor.matmul(
                        psk[0:D, sn * 512 : (sn + 1) * 512],
                        wslice,
                        kt_sb[:, hc * KH + sn * 512 : hc * KH + (sn + 1) * 512],
                        start=(hc == 0), stop=(hc == HC - 1),
                    )
            nc.vector.tensor_scalar_add(kprojT[:], psk[0:D, :], b_sb[:, 1:2])

            # ---- Q projection per qn pair (one psum + one bias op) ----
            def qproj(qn0):
                psq = ps_pool.tile([128, 1024], f32, tag="ps", name=f"psq{qn0}")
                for hc in range(HC):
                    wslice = w_sb[:, hc * D : (hc + 1) * D]
                    for qi in range(2):
                        c0 = (qn0 + qi) * 4 * H + hc * 512
                        nc.tensor.matmul(
                            psq[0:D, qi * 512 : (qi + 1) * 512],
                            wslice,
                            qt_sb[:, c0 : c0 + 512],
                            start=(hc == 0), stop=(hc == HC - 1),
                        )
                nc.vector.tensor_scalar_add(
                    qprojT[:, qn0 * 512 : (qn0 + 2) * 512], psq[0:D, :],
                    b_sb[:, 0:1],
                )

            qproj(0)

            # ---- scores + exp for q-half 0 ----
            e_tiles = {}
            for kc in range(KC):
                sct = ps_pool.tile([128, 1024], f32, tag="ps", name=f"sc0_{kc}")
                for qi in range(2):
                    nc.tensor.matmul(
                        sct[:, qi * 512 : (qi + 1) * 512],
                        kprojT[:, kc * 128 : (kc + 1) * 128],
                        qprojT[:, qi * 512 : (qi + 1) * 512],
                        start=True, stop=True,
                    )
                et = e_pool.tile([128, 1024], bf16, tag="e", name=f"e0_{kc}")
                nc.scalar.activation(et[:], sct[:], AF.Exp)
                e_tiles[(0, kc)] = et

            # ---- V projection as two transient half-sums + vp assembly ----
            psv1 = ps_pool.tile([128, 1024], f32, tag="ps", name="psv1")
            for hc in range(4):
                wslice = w_sb[:, (2 * HC + hc) * D : (2 * HC + hc + 1) * D]
                for sn in range(2):
                    nc.tensor.matmul(
                        psv1[0:D, sn * 512 : (sn + 1) * 512],
                        wslice,
                        vt_sb[:, hc * KH + sn * 512 : hc * KH + (sn + 1) * 512],
                        start=(hc == 0), stop=(hc == 3),
                    )
            nc.vector.tensor_scalar_add(vhalf[:], psv1[0:D, :], b_sb[:, 2:3])
            psv2 = ps_pool.tile([128, 1024], f32, tag="ps", name="psv2")
            for hc in range(4, HC):
                wslice = w_sb[:, (2 * HC + hc) * D : (2 * HC + hc + 1) * D]
                for sn in range(2):
                    nc.tensor.matmul(
                        psv2[0:D, sn * 512 : (sn + 1) * 512],
                        wslice,
                        vt_sb[:, hc * KH + sn * 512 : hc * KH + (sn + 1) * 512],
                        start=(hc == 4), stop=(hc == HC - 1),
                    )
            nc.vector.tensor_tensor(
                out=vprojT[:], in0=psv2[0:D, :], in1=vhalf[:], op=Alu.add,
            )
            pst = ps_pool.tile([128, 1024], f32, tag="ps", name="pst")
            for s in range(KC):
                nc.tensor.transpose(
                    pst[:, s * 128 : s * 128 + D],
                    vprojT[:, s * 128 : (s + 1) * 128],
                    identf[0:D, 0:D],
                )
            for s in range(KC):
                nc.vector.tensor_copy(
                    vp[:, s * (D + 1) : s * (D + 1) + D],
                    pst[:, s * 128 : s * 128 + D],
                )
            nc.vector.memset(vp[:, D :: D + 1], 1.0)

            qproj(2)

            acc = acc_pool.tile([D + 1, L], f32, tag="acc")

            def attnv(qnp, kc):
                et = e_tiles[(qnp, kc)]
                for qi in range(2):
                    qn = qnp * 2 + qi
                    nc.tensor.matmul(
                        acc[:, qn * 512 : (qn + 1) * 512],
                        vp[:, kc * (D + 1) : (kc + 1) * (D + 1)],
                        et[:, qi * 512 : (qi + 1) * 512],
                        start=(kc == 0), stop=(kc == KC - 1),
                        skip_group_check=True,
                    )

            # ---- attnV half 0 + scores/exp/attnV half 1, interleaved ----
            for kc in range(KC):
                attnv(0, kc)
                sct = ps_pool.tile([128, 1024], f32, tag="ps", name=f"sc1_{kc}")
                for qi in range(2):
                    qn = 2 + qi
                    nc.tensor.matmul(
                        sct[:, qi * 512 : (qi + 1) * 512],
                        kprojT[:, kc * 128 : (kc + 1) * 128],
                        qprojT[:, qn * 512 : (qn + 1) * 512],
                        start=True, stop=True,
                    )
                et = e_pool.tile([128, 1024], bf16, tag="e", name=f"e1_{kc}")
                nc.scalar.activation(et[:], sct[:], AF.Exp)
                e_tiles[(1, kc)] = et
                if kc > 0:
                    attnv(1, kc - 1)
            attnv(1, KC - 1)

            # ---- epilogue: per-qn drain of the unnormalized [65, q] result --
            for qn in range(4):
                nc.vector.tensor_copy(
                    outT_sb[:, qn * 512 : (qn + 1) * 512],
                    acc[:, qn * 512 : (qn + 1) * 512],
                )
                nc.sync.dma_start(
                    out=out_d[:, qn * 512 : (qn + 1) * 512],
                    in_=outT_sb[:, qn * 512 : (qn + 1) * 512],
                )

    nc.compile()
    return nc


_NC_CACHE = None


def _get_nc():
    global _NC_CACHE
    if _NC_CACHE is None:
        _NC_CACHE = build_bass()
    return _NC_CACHE


def _make_in_maps(inputs):
    bf16 = ml_dtypes.bfloat16
    q = np.asarray(inputs["query"], np.float32)
    k = np.asarray(inputs["key"], np.float32)
    v = np.asarray(inputs["value"], np.float32)
    Wq = np.asarray(inputs["Wq"], np.float32) * 0.125
    bq = np.asarray(inputs["bq"], np.float32) * 0.125
    Wk = np.asarray(inputs["Wk"], np.float32)
    bk = np.asarray(inputs["bk"], np.float32)
    Wv = np.asarray(inputs["Wv"], np.float32)
    bv = np.asarray(inputs["bv"], np.float32)

    def packw(W):  # [1024, 64] -> [128, 8*64], hc-major per partition
        return W.reshape(HC, 128, D).transpose(1, 0, 2).reshape(128, HC * D)

    wcat = np.concatenate([packw(Wq), packw(Wk), packw(Wv)], axis=1).astype(bf16)
    bcat = np.stack([bq, bk, bv], axis=1).astype(np.float32)

    def tr(x):  # [S, 1024] -> [128, 8*S]: [p, hc*S + s] = x[s, hc*128+p]
        S = x.shape[0]
        return np.ascontiguousarray(
            x.reshape(S, HC, 128).transpose(2, 1, 0)
        ).reshape(128, HC * S).astype(bf16)

    in_maps = []
    for c in range(NCORES):
        b, j = divmod(c, 2)
        qb = q[b]  # [2048, 1024]
        # [p, qn*4096 + hc*512 + s] = qb[qn*512+s, hc*128+p]
        qT = np.ascontiguousarray(
            qb.reshape(4, 512, HC, 128).transpose(3, 0, 2, 1)
        ).reshape(128, 16 * H).astype(bf16)
        kT = tr(k[b, j * KH : (j + 1) * KH])
        vT = tr(v[b, j * KH : (j + 1) * KH])
        in_maps.append({"qt": qT, "kt": kT, "vt": vT, "w": wcat, "b": bcat})
    return in_maps


def kernel(query, key, value, Wq, bq, Wk, bk, Wv, bv):
    from concourse.bass_utils import run_bass_kernel_spmd

    in_maps = _make_in_maps(
        dict(query=query, key=key, value=value, Wq=Wq, bq=bq, Wk=Wk, bk=bk,
             Wv=Wv, bv=bv)
    )
    nc = _get_nc()
    try:
        res = run_bass_kernel_spmd(nc, in_maps, list(range(NCORES)))
    except Exception:
        res = run_bass_kernel_spmd(nc, in_maps, list(range(NCORES)))
    out = np.empty((N, L, D), np.float32)
    for b in range(N):
        o0 = np.asarray(res.results[2 * b]["out"], np.float32)
        o1 = np.asarray(res.results[2 * b + 1]["out"], np.float32)
        num = o0[0:D] + o1[0:D]  # [64, 2048]
        den = o0[D] + o1[D]  # [2048]
        out[b] = (num / den).T
    return out


# revision 10
# speedup vs baseline: 1.2838x; 1.2838x over previous
"""v8: attention head on 8 trn2 NeuronCores, no collectives.

Sharding: core c handles batch b=c//2 and K/V-half j=c%2. Each core computes
unnormalized attention of the batch's FULL query block (2048 rows) against its
1024-row K/V half; softmax numerator/denominator halves combine linearly on
the host (out = (numA+numB)/(denA+denB) + bv; the V bias is exact on the host
because attention weights sum to 1).

Host-side preprocessing (layout/dtype only): q/k/v cast to bf16 and
pre-transposed to [H, seq]; the 1/8 score scale folded into Wq/bq.

Device: one serial HWDGE ring feeds w, kT, qn0, qn1, vT, qn2, qn3 in
1MB chunks so projections stream right behind the DMA; PE pre-warmed with
f32 identity matmuls; K/Q bias adds ride the scalar engine (Identity+bias)
off the DVE; exp table preloaded at t=0; scores in [k, q] layout; attnV with
V-natural stationary carrying a ones column (row 64 = softmax denominator);
per-qn epilogue DMAs the unnormalized [65, 512] slices out.
"""

import sys

if "/opt/trn_rl_repo" not in sys.path:
    sys.path.insert(0, "/opt/trn_rl_repo")

import numpy as np
import ml_dtypes

N, L, H, D = 4, 2048, 1024, 64
NCORES = 8
HC = H // 128  # 8 h-chunks
KH = L // 2  # 1024 rows of K/V per core
KC = KH // 128  # 8 k-chunks per core


def build_bass():
    import concourse.mybir as mybir
    from concourse import bacc
    from concourse.masks import make_identity
    from concourse.tile import TileContext

    f32 = mybir.dt.float32
    bf16 = mybir.dt.bfloat16
    AF = mybir.ActivationFunctionType

    nc = bacc.Bacc("TRN2", target_bir_lowering=False, debug=False)
    qt_d = nc.dram_tensor("qt", [128, 16 * H], bf16, kind="ExternalInput").ap()
    kt_d = nc.dram_tensor("kt", [128, 8 * H], bf16, kind="ExternalInput").ap()
    vt_d = nc.dram_tensor("vt", [128, 8 * H], bf16, kind="ExternalInput").ap()
    w_d = nc.dram_tensor("w", [128, 3 * HC * D], bf16, kind="ExternalInput").ap()
    b_d = nc.dram_tensor("b", [D, 2], f32, kind="ExternalInput").ap()
    out_d = nc.dram_tensor("out", [D + 1, L], f32, kind="ExternalOutput").ap()

    with TileContext(nc) as tc:
        with (
            tc.tile_pool(name="io", bufs=1) as io_pool,
            tc.tile_pool(name="proj", bufs=1) as proj_pool,
            tc.tile_pool(name="e", bufs=12) as e_pool,
            tc.tile_pool(name="ps", bufs=2, space="PSUM") as ps_pool,
            tc.tile_pool(name="acc", bufs=1, space="PSUM") as acc_pool,
        ):
            w_sb = io_pool.tile([128, 3 * HC * D], bf16, tag="w")
            b_sb = io_pool.tile([D, 2], f32, tag="b")
            kt_sb = io_pool.tile([128, 8 * H], bf16, tag="kt")
            vt_sb = io_pool.tile([128, 8 * H], bf16, tag="vt")
            qt_sb = io_pool.tile([128, 16 * H], bf16, tag="qt")
            # one serial HWDGE ring: critical chain first, 1MB chunks
            nc.sync.dma_start(out=w_sb[:], in_=w_d[:])
            nc.sync.dma_start(out=b_sb[:], in_=b_d[:])

            def chunked(dst, src, total, nch):
                step = total // nch
                for i in range(nch):
                    nc.sync.dma_start(
                        out=dst[:, i * step : (i + 1) * step],
                        in_=src[:, i * step : (i + 1) * step],
                    )

            chunked(kt_sb, kt_d, 8 * H, 2)
            for qn in range(2):
                chunked(qt_sb[:, qn * 4 * H : (qn + 1) * 4 * H],
                        qt_d[:, qn * 4 * H : (qn + 1) * 4 * H], 4 * H, 2)
            chunked(vt_sb, vt_d, 8 * H, 2)
            for qn in range(2, 4):
                chunked(qt_sb[:, qn * 4 * H : (qn + 1) * 4 * H],
                        qt_d[:, qn * 4 * H : (qn + 1) * 4 * H], 4 * H, 2)

            identf = io_pool.tile([128, 128], f32, tag="identf")
            make_identity(nc, identf[:])

            kprojT = proj_pool.tile([D, KH], bf16, tag="kprojT")
            qprojT = proj_pool.tile([D, L], bf16, tag="qprojT")
            vp = proj_pool.tile([128, KC * (D + 1)], bf16, tag="vp")
            outT_sb = proj_pool.tile([D + 1, L], f32, tag="outT")
            vprojT = proj_pool.tile([D, KH], f32, tag="vprojT")

            # preload the exp table while everything else is still loading
            # (overwritten later by the ones-column memset)
            nc.scalar.activation(vp[0:1, D : D + 1], identf[0:1, 0:1], AF.Exp)

            # ---- K projection (PE warmup matmuls share its psum tile) ----
            psk = ps_pool.tile([128, 1024], f32, tag="ps", name="psk")
            for _ in range(10):
                nc.tensor.matmul(
                    psk[:, 0:128], identf[:], identf[:], start=True, stop=True,
                )
            for hc in range(HC):
                wslice = w_sb[:, (HC + hc) * D : (HC + hc + 1) * D]
                for sn in range(2):
                    nc.tensor.matmul(
                        psk[0:D, sn * 512 : (sn + 1) * 512],
                        wslice,
                        kt_sb[:, hc * KH + sn * 512 : hc * KH + (sn + 1) * 512],
                        start=(hc == 0), stop=(hc == HC - 1),
                    )
            nc.scalar.activation(kprojT[:], psk[0:D, :], AF.Identity,
                                 bias=b_sb[:, 1:2])

            # ---- Q projection per qn pair (qi outer, hc inner) ----
            def qproj(qn0):
                psq = ps_pool.tile([128, 1024], f32, tag="ps", name=f"psq{qn0}")
                for qi in range(2):
                    for hc in range(HC):
                        wslice = w_sb[:, hc * D : (hc + 1) * D]
                        c0 = (qn0 + qi) * 4 * H + hc * 512
                        nc.tensor.matmul(
                            psq[0:D, qi * 512 : (qi + 1) * 512],
                            wslice,
                            qt_sb[:, c0 : c0 + 512],
                            start=(hc == 0), stop=(hc == HC - 1),
                        )
                nc.scalar.activation(
                    qprojT[:, qn0 * 512 : (qn0 + 2) * 512], psq[0:D, :],
                    AF.Identity, bias=b_sb[:, 0:1],
                )

            qproj(0)

            # ---- scores + exp for q-half 0 ----
            e_tiles = {}

            def score_tile(qnp, kc):
                sct = ps_pool.tile([128, 1024], f32, tag="ps",
                                   name=f"sc{qnp}_{kc}")
                for qi in range(2):
                    qn = qnp * 2 + qi
                    nc.tensor.matmul(
                        sct[:, qi * 512 : (qi + 1) * 512],
                        kprojT[:, kc * 128 : (kc + 1) * 128],
                        qprojT[:, qn * 512 : (qn + 1) * 512],
                        start=True, stop=True,
                    )
                et = e_pool.tile([128, 1024], bf16, tag="e", name=f"e{qnp}_{kc}")
                nc.scalar.activation(et[:], sct[:], AF.Exp)
                e_tiles[(qnp, kc)] = et

            for kc in range(KC):
                score_tile(0, kc)

            # ---- V projection (single transient psum, no bias) ----
            psv = ps_pool.tile([128, 1024], f32, tag="ps", name="psv")
            for hc in range(HC):
                wslice = w_sb[:, (2 * HC + hc) * D : (2 * HC + hc + 1) * D]
                for sn in range(2):
                    nc.tensor.matmul(
                        psv[0:D, sn * 512 : (sn + 1) * 512],
                        wslice,
                        vt_sb[:, hc * KH + sn * 512 : hc * KH + (sn + 1) * 512],
                        start=(hc == 0), stop=(hc == HC - 1),
                    )
            nc.vector.tensor_copy(vprojT[:], psv[0:D, :])

            qproj(2)

            # ---- vp assembly: PE transpose + one strided DVE copy ----
            pst = ps_pool.tile([128, 1024], f32, tag="ps", name="pst")
            for s in range(KC):
                nc.tensor.transpose(
                    pst[:, s * 128 : s * 128 + D],
                    vprojT[:, s * 128 : (s + 1) * 128],
                    identf[0:D, 0:D],
                )
            nc.vector.tensor_copy(
                vp[:].rearrange("p (kc dd) -> p kc dd", dd=D + 1)[:, :, 0:D],
                pst[:].rearrange("p (kc s) -> p kc s", s=128)[:, :, 0:D],
            )
            nc.vector.memset(vp[:, D :: D + 1], 1.0)

            acc = acc_pool.tile([D + 1, L], f32, tag="acc")

            def attnv(qnp, kc):
                et = e_tiles[(qnp, kc)]
                for qi in range(2):
                    qn = qnp * 2 + qi
                    nc.tensor.matmul(
                        acc[:, qn * 512 : (qn + 1) * 512],
                        vp[:, kc * (D + 1) : (kc + 1) * (D + 1)],
                        et[:, qi * 512 : (qi + 1) * 512],
                        start=(kc == 0), stop=(kc == KC - 1),
                        skip_group_check=True,
                    )

            # ---- attnV half 0 + scores/exp/attnV half 1, interleaved ----
            for kc in range(KC):
                attnv(0, kc)
                score_tile(1, kc)
                if kc > 0:
                    attnv(1, kc - 1)
            attnv(1, KC - 1)

            # ---- epilogue: per-qn drain of the unnormalized [65, q] result --
            for qn in range(4):
                nc.vector.tensor_copy(
                    outT_sb[:, qn * 512 : (qn + 1) * 512],
                    acc[:, qn * 512 : (qn + 1) * 512],
                )
                nc.sync.dma_start(
                    out=out_d[:, qn * 512 : (qn + 1) * 512],
                    in_=outT_sb[:, qn * 512 : (qn + 1) * 512],
                )

    nc.compile()
    return nc


_NC_CACHE = None


def _get_nc():
    global _NC_CACHE
    if _NC_CACHE is None:
        _NC_CACHE = build_bass()
    return _NC_CACHE


def _make_in_maps(inputs):
    bf16 = ml_dtypes.bfloat16
    q = np.asarray(inputs["query"], np.float32)
    k = np.asarray(inputs["key"], np.float32)
    v = np.asarray(inputs["value"], np.float32)
    Wq = np.asarray(inputs["Wq"], np.float32) * 0.125
    bq = np.asarray(inputs["bq"], np.float32) * 0.125
    Wk = np.asarray(inputs["Wk"], np.float32)
    bk = np.asarray(inputs["bk"], np.float32)
    Wv = np.asarray(inputs["Wv"], np.float32)

    def packw(W):  # [1024, 64] -> [128, 8*64], hc-major per partition
        return W.reshape(HC, 128, D).transpose(1, 0, 2).reshape(128, HC * D)

    wcat = np.concatenate([packw(Wq), packw(Wk), packw(Wv)], axis=1).astype(bf16)
    bcat = np.stack([bq, bk], axis=1).astype(np.float32)

    def tr(x):  # [S, 1024] -> [128, 8*S]: [p, hc*S + s] = x[s, hc*128+p]
        S = x.shape[0]
        return np.ascontiguousarray(
            x.reshape(S, HC, 128).transpose(2, 1, 0)
        ).reshape(128, HC * S).astype(bf16)

    in_maps = []
    for c in range(NCORES):
        b, j = divmod(c, 2)
        qb = q[b]  # [2048, 1024]
        # [p, qn*4096 + hc*512 + s] = qb[qn*512+s, hc*128+p]
        qT = np.ascontiguousarray(
            qb.reshape(4, 512, HC, 128).transpose(3, 0, 2, 1)
        ).reshape(128, 16 * H).astype(bf16)
        kT = tr(k[b, j * KH : (j + 1) * KH])
        vT = tr(v[b, j * KH : (j + 1) * KH])
        in_maps.append({"qt": qT, "kt": kT, "vt": vT, "w": wcat, "b": bcat})
    return in_maps


def kernel(query, key, value, Wq, bq, Wk, bk, Wv, bv):
    from concourse.bass_utils import run_bass_kernel_spmd

    in_maps = _make_in_maps(
        dict(query=query, key=key, value=value, Wq=Wq, bq=bq, Wk=Wk, bk=bk,
             Wv=Wv, bv=bv)
    )
    nc = _get_nc()
    try:
        res = run_bass_kernel_spmd(nc, in_maps, list(range(NCORES)))
    except Exception:
        res = run_bass_kernel_spmd(nc, in_maps, list(range(NCORES)))
    bvf = np.asarray(bv, np.float32)
    out = np.empty((N, L, D), np.float32)
    for b in range(N):
        o0 = np.asarray(res.results[2 * b]["out"], np.float32)
        o1 = np.asarray(res.results[2 * b + 1]["out"], np.float32)
        num = o0[0:D] + o1[0:D]  # [64, 2048]
        den = o0[D] + o1[D]  # [2048]
        out[b] = (num / den).T + bvf
    return out
